# revision 1
# baseline (speedup 1.0000x reference)
"""GCNN (2x GraphConv + mean-pool + MLP) on 8 Trainium2 NeuronCores.

Sharding: nodes are split 12500/core; each core owns the edges pointing INTO
its nodes (dst-sharding).  Host-side prep re-orders each core's local nodes by
in-degree into 98 tiles of 128 nodes (padded-CSR with per-tile padded degree),
and builds flat per-edge-slot gather indices into a globally permuted node
table.  On device, each layer is: batched indirect-DMA gather of source rows
(bf16), DVE scale by edge weight, DVE strided segmented reduce, then small PE
matmuls (aggr @ W_rel + x @ W_root) + ReLU.  Layer-1 output is AllGathered
(bf16) to form layer-2's gather table.  Mean-pool partials are computed with
per-tile one-hot matmuls accumulated in PSUM and AllReduced; the tiny MLP runs
replicated on every core.
"""

import os
import numpy as np
import ml_dtypes

import concourse.bass as bass
import concourse.bacc as bacc
import concourse.mybir as mybir
import concourse.tile as tile
from concourse import bass_utils
from concourse.masks import make_identity

BF16 = ml_dtypes.bfloat16

# Problem shape (hardcoded per contest contract).
N = 100000          # nodes
E = 1600000         # edges
F = 32              # input features
H = 64              # hidden features
G = 64              # graphs
W = 8               # cores
NL = N // W         # local nodes per core
P = 128             # partitions
NT = (NL + P - 1) // P   # node tiles per core (98)
NLP = NT * P             # padded local nodes (12544)
NTAB = W * NLP           # permuted global table rows

CHUNK_SLOT_BUDGET = 200  # padded-degree slots per msg buffer chunk


# --------------------------------------------------------------------------
# Host-side prep
# --------------------------------------------------------------------------

def _prep(x, edge_attr, edge_index, batch):
    src = np.asarray(edge_index[0], dtype=np.int64)
    dst = np.asarray(edge_index[1], dtype=np.int64)
    ew = np.asarray(edge_attr, dtype=np.float32)
    batch = np.asarray(batch, dtype=np.int64)
    x = np.asarray(x, dtype=np.float32)

    owner = dst // NL

    pos_all = np.empty(N, dtype=np.int64)      # old global -> position in core
    degs_sorted = np.zeros((W, NLP), dtype=np.int64)
    order_all = np.empty((W, NL), dtype=np.int64)
    for r in range(W):
        m = owner == r
        d_l = dst[m] - r * NL
        deg = np.bincount(d_l, minlength=NL)
        order = np.argsort(deg, kind="stable")
        pos = np.empty(NL, dtype=np.int64)
        pos[order] = np.arange(NL)
        pos_all[r * NL:(r + 1) * NL] = pos
        degs_sorted[r, :NL] = deg[order]
        order_all[r] = order

    tile_deg = degs_sorted.reshape(W, NT, P).max(axis=2)      # [W, NT]
    deg_pad = np.maximum(tile_deg.max(axis=0), 1)             # [NT]
    S = int(deg_pad.sum())
    offs = np.zeros(NT + 1, dtype=np.int64)
    offs[1:] = np.cumsum(deg_pad)

    # old global id -> permuted table row
    gp = np.empty(N, dtype=np.int64)
    for r in range(W):
        gp[r * NL:(r + 1) * NL] = r * NLP + pos_all[r * NL:(r + 1) * NL]

    x_tab = np.zeros((NTAB, F), dtype=BF16)
    x_tab[gp] = x.astype(BF16)

    idx_arr = np.zeros((W, P, S), dtype=np.int32)
    ew_arr = np.zeros((W, P, S), dtype=BF16)
    goh = np.zeros((W, P, NT * G), dtype=BF16)
    xT = np.zeros((W, F, NLP), dtype=BF16)
    for r in range(W):
        m = owner == r
        q = pos_all[dst[m]]                   # position of dst within core
        o2 = np.argsort(q, kind="stable")
        q_s = q[o2]
        src_s = gp[src[m][o2]].astype(np.int32)
        ew_s = ew[m][o2]
        counts = degs_sorted[r]
        starts = np.zeros(NLP + 1, dtype=np.int64)
        starts[1:] = np.cumsum(counts)
        k = np.arange(q_s.size, dtype=np.int64) - starts[q_s]
        t = q_s // P
        p = q_s % P
        col = offs[t] + k
        idx_arr[r, p, col] = src_s
        ew_arr[r, p, col] = ew_s.astype(BF16)

        # graph one-hot (includes the pooling "count" contributions)
        bq = batch[r * NL + order_all[r]]     # [NL] graph id per position
        qq = np.arange(NL, dtype=np.int64)
        goh[r, qq % P, (qq // P) * G + bq] = BF16(1.0)

        xT[r] = x_tab[r * NLP:(r + 1) * NLP].T

    # chunk tiles for gather calls
    chunks = []  # (t0, t1, slot_off, slots)
    t0 = 0
    while t0 < NT:
        t1 = t0
        slots = 0
        while t1 < NT and (t1 == t0 or slots + deg_pad[t1] <= CHUNK_SLOT_BUDGET):
            slots += deg_pad[t1]
            t1 += 1
        chunks.append((t0, t1, int(offs[t0]), int(slots)))
        t0 = t1

    meta = {
        "deg_pad": [int(d) for d in deg_pad],
        "offs": [int(o) for o in offs],
        "S": S,
        "chunks": chunks,
        "max_chunk_slots": max(c[3] for c in chunks),
    }
    percore = {
        "idx": idx_arr,
        "ew": ew_arr,
        "goh": goh,
        "xT": xT,
    }
    return meta, percore, x_tab


# --------------------------------------------------------------------------
# Device program
# --------------------------------------------------------------------------

def _build(meta, weights_meta, single_core=False, debug_taps=False):
    """Build the Bass program. weights_meta: dict of flags (has_b1 etc.).

    single_core=True replaces the collectives with plain DMAs (same local
    work) so the program can run under TimelineSim for cost analysis.
    """
    deg_pad = meta["deg_pad"]
    offs = meta["offs"]
    S = meta["S"]
    chunks = meta["chunks"]

    nc = bacc.Bacc("TRN2", target_bir_lowering=False, debug=False,
                   enable_asserts=False,
                   num_devices=(1 if single_core else W))
    f32 = mybir.dt.float32
    bf16 = mybir.dt.bfloat16
    i32 = mybir.dt.int32

    # kernel I/O
    t_xtab = nc.dram_tensor("x_tab", [NTAB, F], bf16, kind="ExternalInput")
    t_idx = nc.dram_tensor("idx", [P, S], i32, kind="ExternalInput")
    t_ew = nc.dram_tensor("ew", [P, S], bf16, kind="ExternalInput")
    t_goh = nc.dram_tensor("goh", [P, NT * G], bf16, kind="ExternalInput")
    t_xT = nc.dram_tensor("xT", [F, NLP], bf16, kind="ExternalInput")
    t_w1r = nc.dram_tensor("w1r", [F, H], bf16, kind="ExternalInput")
    t_w1o = nc.dram_tensor("w1o", [F, H], bf16, kind="ExternalInput")
    t_w2r = nc.dram_tensor("w2r", [H, H], bf16, kind="ExternalInput")
    t_w2o = nc.dram_tensor("w2o", [H, H], bf16, kind="ExternalInput")
    t_lw1 = nc.dram_tensor("lw1", [H, 16], f32, kind="ExternalInput")
    t_lw2 = nc.dram_tensor("lw2", [16, 1], f32, kind="ExternalInput")
    t_b1 = nc.dram_tensor("b1b", [P, H], f32, kind="ExternalInput") if weights_meta["has_b1"] else None
    t_b2 = nc.dram_tensor("b2b", [P, H], f32, kind="ExternalInput") if weights_meta["has_b2"] else None
    t_lb1 = nc.dram_tensor("lb1b", [G, 16], f32, kind="ExternalInput") if weights_meta["has_lb1"] else None
    t_lb2 = nc.dram_tensor("lb2b", [G, 1], f32, kind="ExternalInput") if weights_meta["has_lb2"] else None
    t_out = nc.dram_tensor("out", [G, 1], f32, kind="ExternalOutput")

    MC = meta["max_chunk_slots"]
    slots0 = chunks[0][3]
    taps = {}
    if debug_taps:
        taps["tap_msg"] = nc.dram_tensor("tap_msg", [P, slots0 * F], bf16,
                                         kind="ExternalOutput")
        taps["tap_msgs"] = nc.dram_tensor("tap_msgs", [P, slots0 * F], bf16,
                                          kind="ExternalOutput")
        taps["tap_aggr"] = nc.dram_tensor("tap_aggr", [P, H], f32,
                                          kind="ExternalOutput")
        taps["tap_h1"] = nc.dram_tensor("tap_h1", [P, NT * H], bf16,
                                        kind="ExternalOutput")
        taps["tap_h2"] = nc.dram_tensor("tap_h2", [P, NT * H], bf16,
                                        kind="ExternalOutput")
        taps["tap_part"] = nc.dram_tensor("tap_part", [G, H + 1], f32,
                                          kind="ExternalOutput")
        taps["tap_red"] = nc.dram_tensor("tap_red", [G, H + 1], f32,
                                         kind="ExternalOutput")

    with tile.TileContext(nc) as tc:
        with (
            tc.tile_pool(name="const", bufs=1) as cpool,
            tc.tile_pool(name="msg", bufs=2) as mpool,
            tc.tile_pool(name="work", bufs=3) as wpool,
            tc.tile_pool(name="stage", bufs=1) as spool,
            tc.tile_pool(name="psA", bufs=2, space="PSUM") as psA,
            tc.tile_pool(name="psB", bufs=2, space="PSUM") as psB,
            tc.tile_pool(name="psC", bufs=2, space="PSUM") as psC,
            tc.tile_pool(name="psPool", bufs=1, space="PSUM") as psPool,
            tc.tile_pool(name="dram", bufs=1, space="DRAM") as dpool,
        ):
            # ---- constants into SBUF ----
            ident = cpool.tile([P, P], f32)
            make_identity(nc, ident[:])
            idx_sb = cpool.tile([P, S], i32)
            nc.sync.dma_start(idx_sb[:], t_idx[:, :])
            ew_sb = cpool.tile([P, S], bf16)
            nc.sync.dma_start(ew_sb[:], t_ew[:, :])
            goh_sb = cpool.tile([P, NT * G], bf16)
            nc.sync.dma_start(goh_sb[:], t_goh[:, :])
            xT_sb = cpool.tile([F, NLP], bf16)
            nc.sync.dma_start(xT_sb[:], t_xT[:, :])
            w1r_sb = cpool.tile([F, H], bf16)
            nc.sync.dma_start(w1r_sb[:], t_w1r[:, :])
            w1o_sb = cpool.tile([F, H], bf16)
            nc.sync.dma_start(w1o_sb[:], t_w1o[:, :])
            w2r_sb = cpool.tile([H, H], bf16)
            nc.sync.dma_start(w2r_sb[:], t_w2r[:, :])
            w2o_sb = cpool.tile([H, H], bf16)
            nc.sync.dma_start(w2o_sb[:], t_w2o[:, :])
            lw1_sb = cpool.tile([H, 16], f32)
            nc.sync.dma_start(lw1_sb[:], t_lw1[:, :])
            lw2_sb = cpool.tile([16, 1], f32)
            nc.sync.dma_start(lw2_sb[:], t_lw2[:, :])
            ones_sb = cpool.tile([P, 1], bf16)
            nc.vector.memset(ones_sb[:], 1.0)
            b1_sb = b2_sb = lb1_sb = lb2_sb = None
            if t_b1 is not None:
                b1_sb = cpool.tile([P, H], f32)
                nc.sync.dma_start(b1_sb[:], t_b1[:, :])
            if t_b2 is not None:
                b2_sb = cpool.tile([P, H], f32)
                nc.sync.dma_start(b2_sb[:], t_b2[:, :])
            if t_lb1 is not None:
                lb1_sb = cpool.tile([G, 16], f32)
                nc.sync.dma_start(lb1_sb[:], t_lb1[:, :])
            if t_lb2 is not None:
                lb2_sb = cpool.tile([G, 1], f32)
                nc.sync.dma_start(lb2_sb[:], t_lb2[:, :])

            # staging buffers living across the layer loops
            h1_bf = spool.tile([P, NT * H], bf16)    # layer1 out, node-major
            h1T_sb = spool.tile([H, NT * P], bf16)   # layer1 out, transposed
            h2_bf = spool.tile([P, NT * H], bf16)    # layer2 out, node-major

            # DRAM tiles for the collective
            h1_loc = dpool.tile([NLP, H], bf16)
            h1_full = dpool.tile([NTAB, H], bf16, addr_space="Shared")

            def layer(li, fin, table_ap, rootT_sb, wr_sb, wo_sb, b_sb):
                """One GraphConv layer. fin: input feature count."""
                for (t0, t1, soff, slots) in chunks:
                    msg = mpool.tile([P, MC * H], bf16, tag="msg")
                    mv = msg[:, : slots * fin]
                    # gather: one descriptor per edge-slot
                    # HW contract: one dynamic offset per partition per call
                    # (gathers one 128-row slot-column per call).
                    for j in range(slots):
                        nc.gpsimd.indirect_dma_start(
                            out=mv[:, j * fin : (j + 1) * fin],
                            out_offset=None,
                            in_=table_ap,
                            in_offset=bass.IndirectOffsetOnAxis(
                                ap=idx_sb[:, soff + j : soff + j + 1], axis=0
                            ),
                        )
                    if debug_taps and li == 0 and t0 == 0:
                        nc.sync.dma_start(taps["tap_msg"][:, :], mv)
                    # scale by edge weight (broadcast along features)
                    ew_b = (
                        ew_sb[:, soff : soff + slots]
                        .unsqueeze(2)
                        .broadcast_to([P, slots, fin])
                    )
                    nc.vector.tensor_tensor(
                        out=mv.rearrange("p (j f) -> p j f", f=fin),
                        in0=mv.rearrange("p (j f) -> p j f", f=fin),
                        in1=ew_b,
                        op=mybir.AluOpType.mult,
                    )
                    if debug_taps and li == 0 and t0 == 0:
                        nc.sync.dma_start(taps["tap_msgs"][:, :], mv)
                    for t in range(t0, t1):
                        dp = deg_pad[t]
                        co = offs[t] - soff
                        aggr = wpool.tile([P, H], f32, tag="aggr")
                        seg = msg[:, co * fin : (co + dp) * fin]
                        nc.vector.tensor_reduce(
                            out=aggr[:, :fin],
                            in_=seg.rearrange("p (j f) -> p f j", f=fin),
                            axis=mybir.AxisListType.X,
                            op=mybir.AluOpType.add,
                        )
                        if debug_taps and li == 0 and t == 0:
                            nc.sync.dma_start(taps["tap_aggr"][:, :], aggr[:])
                        # aggr^T via PE
                        aggrT_ps = psA.tile([fin, P], f32, tag="aggrT_ps")
                        nc.tensor.transpose(aggrT_ps[:], aggr[:, :fin], ident[:])
                        aggrT = wpool.tile([fin, P], bf16, tag="aggrT")
                        nc.scalar.copy(aggrT[:], aggrT_ps[:])
                        # out = aggr @ Wrel + x @ Wroot
                        o_ps = psB.tile([P, H], f32, tag="o_ps")
                        nc.tensor.matmul(o_ps[:], aggrT[:], wr_sb[:],
                                         start=True, stop=False)
                        nc.tensor.matmul(
                            o_ps[:], rootT_sb[:, t * P : (t + 1) * P], wo_sb[:],
                            start=False, stop=True,
                        )
                        if b_sb is not None:
                            hsum = wpool.tile([P, H], f32, tag="hsum")
                            nc.vector.tensor_add(hsum[:], o_ps[:], b_sb[:])
                            act_in = hsum
                        else:
                            act_in = o_ps
                        if li == 0:
                            h_f32 = wpool.tile([P, H], f32, tag="hf32")
                            nc.scalar.activation(
                                h_f32[:], act_in[:],
                                mybir.ActivationFunctionType.Relu)
                            nc.scalar.activation(
                                h1_bf[:, t * H : (t + 1) * H], act_in[:],
                                mybir.ActivationFunctionType.Relu)
                            hT_ps = psC.tile([H, P], f32, tag="hT_ps")
                            nc.tensor.transpose(hT_ps[:], h_f32[:], ident[:])
                            nc.scalar.copy(h1T_sb[:, t * P : (t + 1) * P],
                                           hT_ps[:])
                        else:
                            nc.scalar.activation(
                                h2_bf[:, t * H : (t + 1) * H], act_in[:],
                                mybir.ActivationFunctionType.Relu)

            # ---- layer 1 ----
            layer(0, F, t_xtab[:, :], xT_sb, w1r_sb, w1o_sb, b1_sb)

            # h1 -> DRAM (bf16) and AllGather into the layer-2 table
            nc.sync.dma_start(
                h1_loc[:].rearrange("(t p) h -> p t h", p=P),
                h1_bf[:].rearrange("p (t h) -> p t h", h=H),
            )
            if single_core:
                nc.sync.dma_start(h1_full[:NLP, :], h1_loc[:])
            else:
                nc.gpsimd.collective_compute(
                    "AllGather",
                    mybir.AluOpType.bypass,
                    replica_groups=[list(range(W))],
                    ins=[h1_loc[:]],
                    outs=[h1_full[:]],
                )

            # ---- layer 2 ----
            layer(1, H, h1_full[:], h1T_sb, w2r_sb, w2o_sb, b2_sb)

            if debug_taps:
                nc.sync.dma_start(taps["tap_h1"][:, :], h1_bf[:])
                nc.sync.dma_start(taps["tap_h2"][:, :], h2_bf[:])

            # ---- global mean pool (partials) ----
            sums_ps = psPool.tile([G, H], f32)
            cnt_ps = psPool.tile([G, 1], f32)
            for t in range(NT):
                lhs = goh_sb[:, t * G : (t + 1) * G]
                nc.tensor.matmul(sums_ps[:], lhs,
                                 h2_bf[:, t * H : (t + 1) * H],
                                 start=(t == 0), stop=(t == NT - 1))
                nc.tensor.matmul(cnt_ps[:], lhs, ones_sb[:],
                                 start=(t == 0), stop=(t == NT - 1))
            part_sb = wpool.tile([G, H + 1], f32, tag="part")
            nc.scalar.copy(part_sb[:, :H], sums_ps[:])
            nc.scalar.copy(part_sb[:, H : H + 1], cnt_ps[:])

            # AllReduce pooled partials
            pool_in = dpool.tile([G, H + 1], f32)
            pool_out = dpool.tile([G, H + 1], f32, addr_space="Shared")
            nc.sync.dma_start(pool_in[:], part_sb[:])
            if single_core:
                nc.sync.dma_start(pool_out[:], pool_in[:])
            else:
                nc.gpsimd.collective_compute(
                    "AllReduce",
                    mybir.AluOpType.add,
                    replica_groups=[list(range(W))],
                    ins=[pool_in[:]],
                    outs=[pool_out[:]],
                )
            red_sb = wpool.tile([G, H + 1], f32, tag="red")
            nc.sync.dma_start(red_sb[:], pool_out[:])
            if debug_taps:
                nc.sync.dma_start(taps["tap_part"][:, :], part_sb[:])
                nc.sync.dma_start(taps["tap_red"][:, :], red_sb[:])

            # pooled = sums / max(cnt, 1)
            cnt_m = wpool.tile([G, 1], f32, tag="cntm")
            nc.vector.tensor_scalar_max(cnt_m[:], red_sb[:, H : H + 1], 1.0)
            rcnt = wpool.tile([G, 1], f32, tag="rcnt")
            nc.vector.reciprocal(rcnt[:], cnt_m[:])
            pooled = wpool.tile([G, H], f32, tag="pooled")
            nc.vector.tensor_scalar_mul(pooled[:], red_sb[:, :H], rcnt[:, :1])

            # ---- MLP ----
            pT_ps = psA.tile([H, G], f32, tag="aggrT_ps")
            nc.tensor.transpose(pT_ps[:], pooled[:], ident[:G, :G])
            pT_sb = wpool.tile([H, G], f32, tag="pT")
            nc.scalar.copy(pT_sb[:], pT_ps[:])
            m1_ps = psB.tile([G, 16], f32, tag="o_ps")
            nc.tensor.matmul(m1_ps[:], pT_sb[:], lw1_sb[:], start=True, stop=True)
            m1 = wpool.tile([G, 16], f32, tag="m1")
            if lb1_sb is not None:
                nc.vector.tensor_add(m1[:], m1_ps[:], lb1_sb[:])
                nc.scalar.activation(m1[:], m1[:],
                                     mybir.ActivationFunctionType.Relu)
            else:
                nc.scalar.activation(m1[:], m1_ps[:],
                                     mybir.ActivationFunctionType.Relu)
            m1T_ps = psC.tile([16, G], f32, tag="hT_ps")
            nc.tensor.transpose(m1T_ps[:], m1[:], ident[:G, :G])
            m1T = wpool.tile([16, G], f32, tag="m1T")
            nc.scalar.copy(m1T[:], m1T_ps[:])
            o_ps = psA.tile([G, 1], f32, tag="aggrT_ps")
            nc.tensor.matmul(o_ps[:], m1T[:], lw2_sb[:], start=True, stop=True)
            o_sb = wpool.tile([G, 1], f32, tag="osb")
            if lb2_sb is not None:
                nc.vector.tensor_add(o_sb[:], o_ps[:], lb2_sb[:])
            else:
                nc.vector.tensor_copy(o_sb[:], o_ps[:])
            nc.sync.dma_start(t_out[:, :], o_sb[:])

    nc.compile()
    return nc


# --------------------------------------------------------------------------
# Entry point
# --------------------------------------------------------------------------

_CACHE = {}
LAST_RESULTS = None


def kernel(x, edge_attr, w1_rel, b1, w1_root, w2_rel, b2, w2_root,
           lw1, lb1, lw2, lb2, edge_index, batch):
    global LAST_RESULTS
    meta, percore, x_tab = _prep(x, edge_attr, edge_index, batch)

    b1 = np.asarray(b1, dtype=np.float32)
    b2 = np.asarray(b2, dtype=np.float32)
    lb1 = np.asarray(lb1, dtype=np.float32)
    lb2 = np.asarray(lb2, dtype=np.float32)
    weights_meta = {
        "has_b1": bool(np.any(b1 != 0)),
        "has_b2": bool(np.any(b2 != 0)),
        "has_lb1": bool(np.any(lb1 != 0)),
        "has_lb2": bool(np.any(lb2 != 0)),
    }

    key = (meta["S"], tuple(meta["deg_pad"]), tuple(sorted(weights_meta.items())))
    nc = _CACHE.get(key)
    if nc is None:
        nc = _build(meta, weights_meta)
        _CACHE[key] = nc

    base = {
        "x_tab": np.ascontiguousarray(x_tab),
        "w1r": np.ascontiguousarray(np.asarray(w1_rel)).astype(BF16),
        "w1o": np.ascontiguousarray(np.asarray(w1_root)).astype(BF16),
        "w2r": np.ascontiguousarray(np.asarray(w2_rel)).astype(BF16),
        "w2o": np.ascontiguousarray(np.asarray(w2_root)).astype(BF16),
        "lw1": np.ascontiguousarray(np.asarray(lw1, dtype=np.float32)),
        "lw2": np.ascontiguousarray(np.asarray(lw2, dtype=np.float32)),
    }
    if weights_meta["has_b1"]:
        base["b1b"] = np.broadcast_to(b1, (P, H)).copy()
    if weights_meta["has_b2"]:
        base["b2b"] = np.broadcast_to(b2, (P, H)).copy()
    if weights_meta["has_lb1"]:
        base["lb1b"] = np.broadcast_to(lb1, (G, 16)).copy()
    if weights_meta["has_lb2"]:
        base["lb2b"] = np.broadcast_to(lb2.reshape(1, 1), (G, 1)).copy()

    in_maps = []
    for r in range(W):
        m = dict(base)
        m["idx"] = np.ascontiguousarray(percore["idx"][r])
        m["ew"] = np.ascontiguousarray(percore["ew"][r])
        m["goh"] = np.ascontiguousarray(percore["goh"][r])
        m["xT"] = np.ascontiguousarray(percore["xT"][r])
        in_maps.append(m)

    trace = bool(int(os.environ.get("KERNEL_TRACE", "0")))
    try:
        res = bass_utils.run_bass_kernel_spmd(
            nc, in_maps, core_ids=list(range(W)), trace=trace,
        )
    except ModuleNotFoundError:
        # axon NTFF profile hook unavailable in this container
        res = bass_utils.run_bass_kernel_spmd(
            nc, in_maps, core_ids=list(range(W)), trace=False,
        )
    LAST_RESULTS = res
    out = np.asarray(res.results[0]["out"], dtype=np.float32).reshape(G, 1)
    return out



# revision 17
# speedup vs baseline: 3.6994x; 3.6994x over previous
"""GCNN (2x GraphConv + mean-pool + MLP) on 8 Trainium2 NeuronCores.

Sharding: nodes split 12500/core by dst block; each core owns the edges into
its nodes.  Node features live in DRAM tables packed 4 nodes per row
(25088 rows), so edge gathers use the custom SWDGE dma_gather instruction
with int16 indices (idx = node>>2) in 4 class streams (class = node&3, each
class gathering at a different byte offset into the row).  Per 128-slot
chunk, a PE matmul  aggrT[:, win] += msg^T @ seg  performs the edge-weighted
segment-sum, where seg = ew * onehot(dstcol) is built on DVE from uploaded
per-slot (dstcol, ew) streams.  Layer outputs feed small PE matmuls
(aggr @ W_rel + x @ W_root) + ReLU; layer-1 output is written back into a
packed table and AllGathered for layer 2.  Mean-pool partials accumulate in
PSUM via per-tile one-hot matmuls and are AllReduced; the tiny MLP runs
replicated.
"""

import os
import numpy as np
import ml_dtypes

import concourse.bass as bass
import concourse.bacc as bacc
import concourse.mybir as mybir
import concourse.tile as tile
from concourse import bass_utils
from concourse.masks import make_identity

BF16 = ml_dtypes.bfloat16

# Problem shape (hardcoded per contest contract).
N = 100000          # nodes
E = 1600000         # edges
F = 32              # input features
H = 64              # hidden features
G = 64              # graphs
W = 8               # cores
NL = N // W         # owned nodes per core (12500)
P = 128             # partitions
NT = 98             # node tiles per core
NLP = NT * P        # padded local nodes (12544)
NTAB = W * NLP      # permuted global table rows (100352)
R4 = NTAB // 4      # packed table rows (4 nodes each, 25088 <= int16 max)
NCLS = 4            # classes = node & 3
JW = 128            # seg window = whole 128-dst tile
GT = 7              # tiles per gather group
NGRP = NT // GT     # 14 groups
CALL_MAX = 1024     # SWDGE ring cap: 64 data descriptors/engine/queue


# --------------------------------------------------------------------------
# Raw dma_gather builder (bass.dma_gather minus the elem%256B assert).
# Per q7_kernels/extended_inst/dma_gather.cpp the non-transpose HBM path
# supports arbitrary elem byte sizes; only the row stride is 256B-quantized.
# --------------------------------------------------------------------------

def _dma_gather_small(gp, out_ap, in_ap, idxs_ap, num_idxs, elem_size,
                      elem_step):
    assert idxs_ap.dtype == mybir.dt.int16
    assert in_ap.dtype == out_ap.dtype
    stride_bytes = elem_step * mybir.dt.size(in_ap.dtype)
    assert stride_bytes % 256 == 0 and stride_bytes // 256 < 256
    assert in_ap.ap[0][0] == elem_step
    assert in_ap.ap[-1][1] == out_ap.ap[-1][1] == elem_size
    assert out_ap.ap[0][1] * out_ap.ap[1][1] == ((num_idxs + 127) // 128) * 128

    _in_ap = gp.lower_ap_dma(in_ap, for_custom_bir_dma=True)
    _idxs_ap = gp.lower_ap(idxs_ap)
    _out_ap = gp.lower_ap(out_ap)
    return gp.add_instruction(
        mybir.InstDMAGatherAnt(
            name=gp.bass.get_next_instruction_name(),
            ins=[*_in_ap, _idxs_ap,
                 gp.lower_val_access(gp.to_reg(num_idxs))],
            outs=[_out_ap],
            transpose=False,
            num_idxs=num_idxs,
            elem_size=elem_size,
            stride_bytes_256=stride_bytes // 256,
            gen_mode=0,
            single_packet=True,
            queue_num=0,
            sbuf_tokens_per_rank=0,
            sbuf_free_dim_per_rank=0,
            sbuf_free_dim_pad_per_rank=0,
            sbuf_byte_offset=0,
        )
    )


# --------------------------------------------------------------------------
# Host-side prep
# --------------------------------------------------------------------------

def _cells_layout(M):
    """Static slot/span layout shared by all cores.

    M: [NCLS, NT] exact per-cell slot counts (max over cores).  Stream
    order: (grp, cls, tile-in-grp); each (grp, cls) range is padded to a
    multiple of 128 and split into 128-slot blocks.  Block spans record
    which tile each partition sub-range [a, b) belongs to, so the per-tile
    seg matmuls can slice lhsT/rhs partitions exactly.
    """
    cell_off = np.zeros((NCLS, NT), dtype=np.int64)
    groups = []  # [grp][cls] -> {slot0, nslots, nblk, spans[t] = [(q, a, b)]}
    pos = 0
    for g in range(NGRP):
        gcls = []
        for c in range(NCLS):
            slot0 = pos
            spans = {}
            for t in range(g * GT, (g + 1) * GT):
                cell_off[c, t] = pos
                n = int(M[c, t])
                lo = pos - slot0
                hi = lo + n
                q0, q1 = lo // 128, (hi + 127) // 128
                for q in range(q0, q1):
                    a = max(lo - q * 128, 0)
                    b = min(hi - q * 128, 128)
                    if b > a:
                        spans.setdefault(t, []).append((q, a, b))
                pos += n
            nsl = -(-(pos - slot0) // 128) * 128
            pos = slot0 + nsl
            gcls.append({"slot0": slot0, "nslots": nsl,
                         "nblk": nsl // 128, "spans": spans})
        groups.append(gcls)
    return cell_off, groups, pos


def _prep(x, edge_attr, edge_index, batch):
    src = np.asarray(edge_index[0], dtype=np.int64)
    dst = np.asarray(edge_index[1], dtype=np.int64)
    ew = np.asarray(edge_attr, dtype=np.float32)
    batch = np.asarray(batch, dtype=np.int64)
    x = np.asarray(x, dtype=np.float32)

    owner = dst // NL
    d_loc = dst - owner * NL
    s_own = src // NL
    g_src = s_own * NLP + (src - s_own * NL)       # permuted global src id
    idx16 = (g_src >> 2).astype(np.int16)
    cls = (g_src & 3).astype(np.int64)
    tile_e = d_loc >> 7

    # per (core, cls, tile) counts -> shared exact cell sizes M
    key = (owner * NCLS + cls) * NT + tile_e
    counts = np.bincount(key, minlength=W * NCLS * NT).reshape(W, NCLS, NT)
    M = counts.max(axis=0)

    cell_off, groups, NSLOT = _cells_layout(M)
    NCH = NSLOT // 128

    QMAX = max(len(gc["chunks"]) for grp in groups for gc in grp)
    idx_arr = np.zeros((W, 128, NSLOT // 16), dtype=np.int16)
    ew_arr = np.zeros((W, 128, NCH + QMAX), dtype=BF16)
    dstrel_arr = np.zeros((W, 128, NCH + QMAX), dtype=BF16)
    goh = np.zeros((W, P, NT * G), dtype=BF16)
    xT = np.zeros((W, F, NLP), dtype=BF16)
    t1 = np.zeros((R4, 4 * F), dtype=BF16)

    # packed x table (4 nodes per row)
    xp = np.zeros((NTAB, F), dtype=np.float32)
    for r in range(W):
        xp[r * NLP:r * NLP + NL] = x[r * NL:(r + 1) * NL]
    t1[:] = xp.reshape(R4, 4 * F).astype(BF16)

    cell_off_flat = cell_off.reshape(-1)
    for r in range(W):
        m = owner == r
        e_cls = cls[m]
        e_d = d_loc[m]
        order = np.lexsort((e_d, e_cls))
        e_cls = e_cls[order]
        e_d = e_d[order]
        e_idx16 = idx16[m][order]
        e_ew = ew[m][order]
        e_tile = e_d >> 7
        e_win = (e_d >> 6) & 1
        e_col = e_d & 127
        ckey = (e_cls * NT + e_tile) * NW + e_win   # sorted (runs)
        # rank within cell
        change = np.empty(ckey.size, dtype=bool)
        change[0] = True
        change[1:] = ckey[1:] != ckey[:-1]
        run_starts = np.flatnonzero(change)
        run_ids = np.cumsum(change) - 1
        rank = np.arange(ckey.size) - run_starts[run_ids]
        slot = cell_off_flat[ckey] + rank

        sl_idx = np.zeros(NSLOT, dtype=np.int16)
        sl_ew = np.zeros(NSLOT, dtype=np.float32)
        sl_dr = np.zeros(NSLOT, dtype=np.float32)
        sl_idx[slot] = e_idx16
        sl_ew[slot] = e_ew
        sl_dr[slot] = (e_col - JW * e_win).astype(np.float32)

        idx_arr[r] = np.tile(sl_idx.reshape(NSLOT // 16, 16).T, (8, 1))
        ew_arr[r, :, :NCH] = sl_ew.reshape(NCH, 128).T.astype(BF16)
        dstrel_arr[r, :, :NCH] = sl_dr.reshape(NCH, 128).T.astype(BF16)

        # graph one-hot for pooling + root-path xT (natural node order)
        ln = np.arange(NL, dtype=np.int64)
        bq = batch[r * NL + ln]
        goh[r, ln % P, (ln // P) * G + bq] = BF16(1.0)
        xT[r, :, :NL] = x[r * NL:(r + 1) * NL].T.astype(BF16)

    meta = {
        "M": M,
        "NSLOT": NSLOT,
        "NCH": NCH,
        "QMAX": QMAX,
        "groups": groups,
    }
    percore = {
        "idx": idx_arr,
        "ew": ew_arr,
        "dstrel": dstrel_arr,
        "goh": goh,
        "xT": xT,
    }
    return meta, percore, t1


# --------------------------------------------------------------------------
# Device program
# --------------------------------------------------------------------------

def _build(meta, weights_meta, single_core=False):
    NSLOT = meta["NSLOT"]
    NCH = meta["NCH"]
    groups = meta["groups"]
    QMAX = meta["QMAX"]

    nc = bacc.Bacc("TRN2", target_bir_lowering=False, debug=False,
                   enable_asserts=False,
                   num_devices=(1 if single_core else W))
    f32 = mybir.dt.float32
    bf16 = mybir.dt.bfloat16
    i16 = mybir.dt.int16

    t_tab1 = nc.dram_tensor("tab1", [R4, 4 * F], bf16, kind="ExternalInput")
    t_idx = nc.dram_tensor("idx", [128, NSLOT // 16], i16,
                           kind="ExternalInput")
    t_ew = nc.dram_tensor("ew", [128, NCH + QMAX], bf16,
                          kind="ExternalInput")
    t_dr = nc.dram_tensor("dstrel", [128, NCH + QMAX], bf16,
                          kind="ExternalInput")
    t_goh = nc.dram_tensor("goh", [P, NT * G], bf16, kind="ExternalInput")
    t_xT = nc.dram_tensor("xT", [F, NLP], bf16, kind="ExternalInput")
    t_iota = nc.dram_tensor("iotam", [P, JW * QMAX], bf16,
                            kind="ExternalInput")
    t_w1r = nc.dram_tensor("w1r", [F, H], bf16, kind="ExternalInput")
    t_w1o = nc.dram_tensor("w1o", [F, H], bf16, kind="ExternalInput")
    t_w2r = nc.dram_tensor("w2r", [H, H], bf16, kind="ExternalInput")
    t_w2o = nc.dram_tensor("w2o", [H, H], bf16, kind="ExternalInput")
    t_lw1 = nc.dram_tensor("lw1", [H, 16], f32, kind="ExternalInput")
    t_lw2 = nc.dram_tensor("lw2", [16, 1], f32, kind="ExternalInput")
    t_b1 = nc.dram_tensor("b1b", [P, H], f32, kind="ExternalInput") \
        if weights_meta["has_b1"] else None
    t_b2 = nc.dram_tensor("b2b", [P, H], f32, kind="ExternalInput") \
        if weights_meta["has_b2"] else None
    t_lb1 = nc.dram_tensor("lb1b", [G, 16], f32, kind="ExternalInput") \
        if weights_meta["has_lb1"] else None
    t_lb2 = nc.dram_tensor("lb2b", [G, 1], f32, kind="ExternalInput") \
        if weights_meta["has_lb2"] else None
    t_out = nc.dram_tensor("out", [G, 1], f32, kind="ExternalOutput")

    with tile.TileContext(nc) as tc:
        with (
            tc.tile_pool(name="const", bufs=1) as cpool,
            tc.tile_pool(name="msg", bufs=2) as mpool,
            tc.tile_pool(name="seg", bufs=2) as segpool,
            tc.tile_pool(name="work", bufs=3) as wpool,
            tc.tile_pool(name="xtg", bufs=2) as xtgpool,
            tc.tile_pool(name="psA", bufs=2, space="PSUM") as psA,
            tc.tile_pool(name="psB", bufs=2, space="PSUM") as psB,
            tc.tile_pool(name="psC", bufs=2, space="PSUM") as psC,
            tc.tile_pool(name="psPool", bufs=1, space="PSUM") as psPool,
            tc.tile_pool(name="dram", bufs=1, space="DRAM") as dpool,
        ):
            # ---- constants ----
            ident = cpool.tile([P, P], f32)
            make_identity(nc, ident[:])
            ident_bf = cpool.tile([P, P], bf16)
            make_identity(nc, ident_bf[:])
            idx_sb = cpool.tile([128, NSLOT // 16], i16)
            nc.sync.dma_start(idx_sb[:], t_idx[:, :])
            ew_sb = cpool.tile([128, NCH + QMAX], bf16)
            nc.sync.dma_start(ew_sb[:], t_ew[:, :])
            dr_sb = cpool.tile([128, NCH + QMAX], bf16)
            nc.sync.dma_start(dr_sb[:], t_dr[:, :])
            goh_sb = cpool.tile([P, NT * G], bf16)
            nc.sync.dma_start(goh_sb[:], t_goh[:, :])
            iota_sb = cpool.tile([P, JW * QMAX], bf16)
            nc.sync.dma_start(iota_sb[:], t_iota[:, :])
            w1r_sb = cpool.tile([F, H], bf16)
            nc.sync.dma_start(w1r_sb[:], t_w1r[:, :])
            w1o_sb = cpool.tile([F, H], bf16)
            nc.sync.dma_start(w1o_sb[:], t_w1o[:, :])
            w2r_sb = cpool.tile([H, H], bf16)
            nc.sync.dma_start(w2r_sb[:], t_w2r[:, :])
            w2o_sb = cpool.tile([H, H], bf16)
            nc.sync.dma_start(w2o_sb[:], t_w2o[:, :])
            lw1_sb = cpool.tile([H, 16], f32)
            nc.sync.dma_start(lw1_sb[:], t_lw1[:, :])
            lw2_sb = cpool.tile([16, 1], f32)
            nc.sync.dma_start(lw2_sb[:], t_lw2[:, :])
            ones_sb = cpool.tile([P, 1], bf16)
            nc.vector.memset(ones_sb[:], 1.0)
            zeros_sb = cpool.tile([P, P], bf16)
            nc.vector.memset(zeros_sb[:], 0.0)
            b1_sb = b2_sb = lb1_sb = lb2_sb = None
            if t_b1 is not None:
                b1_sb = cpool.tile([P, H], f32)
                nc.sync.dma_start(b1_sb[:], t_b1[:, :])
            if t_b2 is not None:
                b2_sb = cpool.tile([P, H], f32)
                nc.sync.dma_start(b2_sb[:], t_b2[:, :])
            if t_lb1 is not None:
                lb1_sb = cpool.tile([G, 16], f32)
                nc.sync.dma_start(lb1_sb[:], t_lb1[:, :])
            if t_lb2 is not None:
                lb2_sb = cpool.tile([G, 1], f32)
                nc.sync.dma_start(lb2_sb[:], t_lb2[:, :])

            # layer-1 output staging (consumed by layer 2 root path)
            h1_bf = cpool.tile([P, NT * H], bf16)

            # DRAM tiles for the collective
            t2_loc = dpool.tile([NLP // 4, 4 * H], bf16)
            t2_full = dpool.tile([R4, 4 * H], bf16, addr_space="Shared")

            sums_ps = psPool.tile([G, H], f32)
            cnt_ps = psPool.tile([G, 1], f32)

            def layer(li, fin, table, wr_sb, wo_sb, b_sb):
                row_elems = 4 * fin
                for g in range(NGRP):
                    if li == 0:
                        xTg = xtgpool.tile([F, GT * P], bf16, tag="xtg")
                        nc.sync.dma_start(
                            xTg[:], t_xT[:, g * GT * P:(g + 1) * GT * P])
                    # gather 4 class streams + build seg matrices
                    msgs = []
                    segs = []
                    for c in range(NCLS):
                        gc = groups[g][c]
                        nsl = gc["nslots"]
                        q0 = gc["slot0"] // 128
                        nq = len(gc["chunks"])
                        msg = mpool.tile([P, (QMAX + 1) * H], bf16,
                                         tag=f"msg{c}")
                        # SWDGE ring caps one call at 1024 idxs (64 data
                        # descriptors per engine per queue).
                        for s0 in range(0, nsl, 1024):
                            n1 = min(1024, nsl - s0)
                            b0 = s0 // 128
                            a0 = gc["slot0"] + s0
                            _dma_gather_small(
                                nc.gpsimd,
                                out_ap=msg[:, b0 * fin:
                                           b0 * fin + (n1 // 128) * fin]
                                .rearrange("p (b e) -> p b e", e=fin),
                                in_ap=table[:, c * fin:(c + 1) * fin],
                                idxs_ap=idx_sb[:, a0 // 16:(a0 + n1) // 16],
                                num_idxs=n1,
                                elem_size=fin,
                                elem_step=row_elems,
                            )
                        seg = segpool.tile([P, JW * QMAX], bf16,
                                           tag=f"seg{c}")
                        if nq:
                            sv = seg[:].rearrange(
                                "p (j q) -> p j q", q=QMAX)
                            nc.vector.tensor_tensor(
                                out=sv,
                                in0=iota_sb[:].rearrange(
                                    "p (j q) -> p j q", q=QMAX),
                                in1=dr_sb[:, q0:q0 + QMAX].unsqueeze(1)
                                .broadcast_to([P, JW, QMAX]),
                                op=mybir.AluOpType.is_equal,
                            )
                            nc.vector.tensor_tensor(
                                out=sv,
                                in0=sv,
                                in1=ew_sb[:, q0:q0 + QMAX].unsqueeze(1)
                                .broadcast_to([P, JW, QMAX]),
                                op=mybir.AluOpType.mult,
                            )
                        msgs.append(msg)
                        segs.append(seg)

                    for tt in range(GT):
                        t = g * GT + tt
                        ps = psA.tile([fin, P], f32, tag="aggrT")
                        nc.tensor.matmul(ps[:], zeros_sb[:, :fin],
                                         zeros_sb[:], start=True, stop=False)
                        for c in range(NCLS):
                            gc = groups[g][c]
                            nq = len(gc["chunks"])
                            for qq, (ct, cw) in enumerate(gc["chunks"]):
                                if ct != t:
                                    continue
                                nc.tensor.matmul(
                                    ps[:, cw * JW:(cw + 1) * JW],
                                    msgs[c][:, qq * fin:(qq + 1) * fin],
                                    segs[c][:].rearrange(
                                        "p (j q) -> p j q", q=QMAX)
                                    [:, :, qq:qq + 1].rearrange(
                                        "p j one -> p (j one)"),
                                    start=False, stop=False,
                                )
                        nc.tensor.matmul(ps[:], zeros_sb[:, :fin],
                                         zeros_sb[:], start=False, stop=True)
                        aggrT = wpool.tile([fin, P], bf16, tag="aggrT_sb")
                        nc.scalar.copy(aggrT[:], ps[:])

                        if li == 0:
                            rootT = xTg[:, tt * P:(tt + 1) * P]
                        else:
                            hT_ps = psC.tile([H, P], bf16, tag="hT")
                            nc.tensor.transpose(
                                hT_ps[:], h1_bf[:, t * H:(t + 1) * H],
                                ident_bf[:])
                            rootT_sb = wpool.tile([H, P], bf16, tag="hT_sb")
                            nc.scalar.copy(rootT_sb[:], hT_ps[:])
                            rootT = rootT_sb[:]

                        o_ps = psB.tile([P, H], f32, tag="o_ps")
                        nc.tensor.matmul(o_ps[:], aggrT[:], wr_sb[:],
                                         start=True, stop=False)
                        nc.tensor.matmul(o_ps[:], rootT, wo_sb[:],
                                         start=False, stop=True)
                        if b_sb is not None:
                            hsum = wpool.tile([P, H], f32, tag="hsum")
                            nc.vector.tensor_add(hsum[:], o_ps[:], b_sb[:])
                            act_in = hsum
                        else:
                            act_in = o_ps
                        if li == 0:
                            nc.scalar.activation(
                                h1_bf[:, t * H:(t + 1) * H], act_in[:],
                                mybir.ActivationFunctionType.Relu)
                            nc.sync.dma_start(
                                t2_loc[t * 32:(t + 1) * 32, :].rearrange(
                                    "r (c f) -> (r c) f", f=H),
                                h1_bf[:, t * H:(t + 1) * H])
                        else:
                            h2t = wpool.tile([P, H], bf16, tag="h2t")
                            nc.scalar.activation(
                                h2t[:], act_in[:],
                                mybir.ActivationFunctionType.Relu)
                            lhs = goh_sb[:, t * G:(t + 1) * G]
                            nc.tensor.matmul(sums_ps[:], lhs, h2t[:],
                                             start=(t == 0),
                                             stop=(t == NT - 1))
                            nc.tensor.matmul(cnt_ps[:], lhs, ones_sb[:],
                                             start=(t == 0),
                                             stop=(t == NT - 1))

            # ---- layer 1 ----
            layer(0, F, t_tab1[:, :], w1r_sb, w1o_sb, b1_sb)

            # AllGather the packed h1 table
            if single_core:
                nc.sync.dma_start(t2_full[:NLP // 4, :], t2_loc[:])
            else:
                nc.gpsimd.collective_compute(
                    "AllGather",
                    mybir.AluOpType.bypass,
                    replica_groups=[list(range(W))],
                    ins=[t2_loc[:]],
                    outs=[t2_full[:]],
                )

            # ---- layer 2 (+ pooling partials) ----
            layer(1, H, t2_full[:, :], w2r_sb, w2o_sb, b2_sb)

            part_sb = wpool.tile([G, H + 1], f32, tag="part")
            nc.scalar.copy(part_sb[:, :H], sums_ps[:])
            nc.scalar.copy(part_sb[:, H:H + 1], cnt_ps[:])

            pool_in = dpool.tile([G, H + 1], f32)
            pool_out = dpool.tile([G, H + 1], f32, addr_space="Shared")
            nc.sync.dma_start(pool_in[:], part_sb[:])
            if single_core:
                nc.sync.dma_start(pool_out[:], pool_in[:])
            else:
                nc.gpsimd.collective_compute(
                    "AllReduce",
                    mybir.AluOpType.add,
                    replica_groups=[list(range(W))],
                    ins=[pool_in[:]],
                    outs=[pool_out[:]],
                )
            red_sb = wpool.tile([G, H + 1], f32, tag="red")
            nc.sync.dma_start(red_sb[:], pool_out[:])

            # pooled = sums / max(cnt, 1)
            cnt_m = wpool.tile([G, 1], f32, tag="cntm")
            nc.vector.tensor_scalar_max(cnt_m[:], red_sb[:, H:H + 1], 1.0)
            rcnt = wpool.tile([G, 1], f32, tag="rcnt")
            nc.vector.reciprocal(rcnt[:], cnt_m[:])
            pooled = wpool.tile([G, H], f32, tag="pooled")
            nc.vector.tensor_scalar_mul(pooled[:], red_sb[:, :H], rcnt[:, :1])

            # ---- MLP ----
            pT_ps = psA.tile([H, G], f32, tag="aggrT")
            nc.tensor.transpose(pT_ps[:], pooled[:], ident[:G, :G])
            pT_sb = wpool.tile([H, G], f32, tag="pT")
            nc.scalar.copy(pT_sb[:], pT_ps[:])
            m1_ps = psB.tile([G, 16], f32, tag="o_ps")
            nc.tensor.matmul(m1_ps[:], pT_sb[:], lw1_sb[:],
                             start=True, stop=True)
            m1 = wpool.tile([G, 16], f32, tag="m1")
            if lb1_sb is not None:
                nc.vector.tensor_add(m1[:], m1_ps[:], lb1_sb[:])
                nc.scalar.activation(m1[:], m1[:],
                                     mybir.ActivationFunctionType.Relu)
            else:
                nc.scalar.activation(m1[:], m1_ps[:],
                                     mybir.ActivationFunctionType.Relu)
            m1T_ps = psC.tile([16, G], f32, tag="hT")
            nc.tensor.transpose(m1T_ps[:], m1[:], ident[:G, :G])
            m1T = wpool.tile([16, G], f32, tag="m1T")
            nc.scalar.copy(m1T[:], m1T_ps[:])
            o_ps = psA.tile([G, 1], f32, tag="aggrT")
            nc.tensor.matmul(o_ps[:], m1T[:], lw2_sb[:], start=True, stop=True)
            o_sb = wpool.tile([G, 1], f32, tag="osb")
            if lb2_sb is not None:
                nc.vector.tensor_add(o_sb[:], o_ps[:], lb2_sb[:])
            else:
                nc.vector.tensor_copy(o_sb[:], o_ps[:])
            nc.sync.dma_start(t_out[:, :], o_sb[:])

    nc.compile()
    return nc


# --------------------------------------------------------------------------
# Entry point
# --------------------------------------------------------------------------

_CACHE = {}
LAST_RESULTS = None


def kernel(x, edge_attr, w1_rel, b1, w1_root, w2_rel, b2, w2_root,
           lw1, lb1, lw2, lb2, edge_index, batch):
    global LAST_RESULTS
    meta, percore, t1 = _prep(x, edge_attr, edge_index, batch)

    b1 = np.asarray(b1, dtype=np.float32)
    b2 = np.asarray(b2, dtype=np.float32)
    lb1 = np.asarray(lb1, dtype=np.float32)
    lb2 = np.asarray(lb2, dtype=np.float32)
    weights_meta = {
        "has_b1": bool(np.any(b1 != 0)),
        "has_b2": bool(np.any(b2 != 0)),
        "has_lb1": bool(np.any(lb1 != 0)),
        "has_lb2": bool(np.any(lb2 != 0)),
    }

    key = (meta["NSLOT"], tuple(meta["M"].reshape(-1).tolist()),
           tuple(sorted(weights_meta.items())))
    nc = _CACHE.get(key)
    if nc is None:
        nc = _build(meta, weights_meta)
        _CACHE[key] = nc

    QMAX = max(len(gc["chunks"]) for grp in meta["groups"] for gc in grp)
    iota_m = np.broadcast_to(
        np.tile(np.arange(JW, dtype=np.float32)[:, None], (1, QMAX))
        .reshape(1, JW * QMAX), (P, JW * QMAX)).astype(BF16)

    base = {
        "tab1": np.ascontiguousarray(t1),
        "iotam": np.ascontiguousarray(iota_m),
        "w1r": np.ascontiguousarray(np.asarray(w1_rel)).astype(BF16),
        "w1o": np.ascontiguousarray(np.asarray(w1_root)).astype(BF16),
        "w2r": np.ascontiguousarray(np.asarray(w2_rel)).astype(BF16),
        "w2o": np.ascontiguousarray(np.asarray(w2_root)).astype(BF16),
        "lw1": np.ascontiguousarray(np.asarray(lw1, dtype=np.float32)),
        "lw2": np.ascontiguousarray(np.asarray(lw2, dtype=np.float32)),
    }
    if weights_meta["has_b1"]:
        base["b1b"] = np.broadcast_to(b1, (P, H)).copy()
    if weights_meta["has_b2"]:
        base["b2b"] = np.broadcast_to(b2, (P, H)).copy()
    if weights_meta["has_lb1"]:
        base["lb1b"] = np.broadcast_to(lb1, (G, 16)).copy()
    if weights_meta["has_lb2"]:
        base["lb2b"] = np.broadcast_to(lb2.reshape(1, 1), (G, 1)).copy()

    in_maps = []
    for r in range(W):
        m = dict(base)
        m["idx"] = np.ascontiguousarray(percore["idx"][r])
        m["ew"] = np.ascontiguousarray(percore["ew"][r])
        m["dstrel"] = np.ascontiguousarray(percore["dstrel"][r])
        m["goh"] = np.ascontiguousarray(percore["goh"][r])
        m["xT"] = np.ascontiguousarray(percore["xT"][r])
        in_maps.append(m)

    trace = bool(int(os.environ.get("KERNEL_TRACE", "0")))
    try:
        res = bass_utils.run_bass_kernel_spmd(
            nc, in_maps, core_ids=list(range(W)), trace=trace,
        )
    except ModuleNotFoundError:
        res = bass_utils.run_bass_kernel_spmd(
            nc, in_maps, core_ids=list(range(W)), trace=False,
        )
    LAST_RESULTS = res
    out = np.asarray(res.results[0]["out"], dtype=np.float32).reshape(G, 1)
    return out
